# revision 13
# baseline (speedup 1.0000x reference)
"""CNN+LSTM seq2seq kernel for trn2, 8-core data parallel — bf16 redesign.

Model (per core, batch 64):
  conv1d(16->64, k=5, same) -> relu -> maxpool2 -> LSTM(64->512) over 512 steps
  -> autoregressive LSTM(1->512) decoder 64 steps with linear head(512->1).

Sharding: batch 512 split across 8 cores; weights replicated. No collectives.

v2 design notes (vs fp32r baseline):
 - all matmul operands bf16: fp32r is 4x slower on LDWEIGHTS and blocks
   column tiling (XBUS budget); bf16 streams 1 cyc/row at any N.
 - gates computed column-STACKED: each gate's [64,512] lives as [128,256]
   (cols 256:512 on partitions 64:128) via two concurrent col-group
   matmuls (tile_position (0,0)/(0,64)) sharing one hT stationary. This
   halves PE stream time AND gives full-128-partition elementwise.
 - gate col order [f|i|o|g] so one sigmoid covers psum[:, 0:768].
 - h produced stacked [128,256] bf16; hT chunks via 2 concurrent
   row-group transpose pairs (base 0/64) straight from the stacked tile.
 - decoder: pred feedback folded into recurrent weights (dWhh' =
   dWhh.T + head_w (x) dWih, db' = db + head_b*dWih); pred itself via
   K=128 N=1 matmuls off the critical path. Step 0 corrected with
   (dstart - pred_enc) (x) dWih.
 - conv: batch pairs via concurrent col-group matmuls (two batches per
   stream pass), bf16 end-to-end, enc_x staged in DRAM as bf16.
"""

import numpy as np

import concourse.bass as bass
import concourse.mybir as mybir
import concourse.tile as tile_mod
from concourse import bacc
from concourse.masks import make_identity

F32 = mybir.dt.float32
BF16 = mybir.dt.bfloat16
AF = mybir.ActivationFunctionType

B = 64        # batch per core
S = 1024      # input seq len
CIN = 16
OC = 64       # conv out channels
KW = 5
T2 = 512      # encoder steps after pool
H = 512       # hidden
G = 4 * H     # gates
OUT_STEPS = 64
NCORES = 8
NXB = 4       # x-slot rotation depth

# my gate col blocks [f|i|o|g]; torch rows are [i|f|g|o].
# PERM2N[jb] = my 128-col block for torch row-block jb (involution).
PERM2N = [4, 5, 6, 7, 0, 1, 2, 3, 12, 13, 14, 15, 8, 9, 10, 11]
# hT tile col offset for h-chunk kc (tile holds [kc0|kc1|kc2|kc3])
CMAP = [0, 64, 128, 192]

PHASES = {}


def _mark(nc, name):
    PHASES[name] = int(nc.get_next_instruction_name().split("-")[-1])


def build_nc(t2=T2, steps=OUT_STEPS, conv_pairs=B // 2):
    nc = bacc.Bacc(None, target_bir_lowering=False, debug=False)

    # ---------- DRAM I/O ----------
    x_d = nc.dram_tensor("x", [B, S, CIN], F32, kind="ExternalInput")
    dstart_d = nc.dram_tensor("decoder_start", [B, 1], F32, kind="ExternalInput")
    convw_d = nc.dram_tensor("conv_w", [OC, CIN, KW], F32, kind="ExternalInput")
    convb_d = nc.dram_tensor("conv_b", [OC], F32, kind="ExternalInput")
    encWih_d = nc.dram_tensor("enc_Wih", [G, OC], F32, kind="ExternalInput")
    encWhh_d = nc.dram_tensor("enc_Whh", [G, H], F32, kind="ExternalInput")
    encb_d = nc.dram_tensor("enc_b", [G], F32, kind="ExternalInput")
    decWih_d = nc.dram_tensor("dec_Wih", [G, 1], F32, kind="ExternalInput")
    decWhh_d = nc.dram_tensor("dec_Whh", [G, H], F32, kind="ExternalInput")
    decb_d = nc.dram_tensor("dec_b", [G], F32, kind="ExternalInput")
    headw_d = nc.dram_tensor("head_w", [1, H], F32, kind="ExternalInput")
    headb_d = nc.dram_tensor("head_b", [1], F32, kind="ExternalInput")
    out_d = nc.dram_tensor("out", [B, OUT_STEPS], F32, kind="ExternalOutput")

    with tile_mod.TileContext(nc) as tc:
        with tc.tile_pool(name="dram", bufs=1, space="DRAM") as dramp:
            enc_x = dramp.tile([T2, B, OC], BF16)   # pooled conv out [t, b, oc]

            with tc.tile_pool(name="const", bufs=1) as cn:
                identity = cn.tile([128, 128], F32)
                make_identity(nc, identity)
                idbf = cn.tile([128, 128], BF16)
                nc.vector.tensor_copy(idbf, identity)

                # persistent weights (all bf16)
                hW = [cn.tile([128, G], BF16, name=f"hW{k}") for k in range(4)]
                xW = cn.tile([OC + 1, G], BF16)       # rows 0:64 Wih.T, row 64 enc_b
                dhW = [cn.tile([128, G], BF16, name=f"dhW{k}") for k in range(4)]
                dbW = cn.tile([1, G], BF16)           # db' = db + head_b*dWih
                dxW0 = cn.tile([1, G], BF16)          # dWih (step-0 correction rhs)
                cwT = cn.tile([CIN, KW * OC], BF16)   # conv taps [16, 5*64]
                cb2 = cn.tile([128, 1], F32)          # conv bias stacked twice
                ones_row = cn.tile([1, B], BF16)
                hbr = cn.tile([1, 1], BF16)           # head_b
                hwT = cn.tile([128, 4], BF16)         # head_w chunks as cols
                zpadb = cn.tile([CIN, 2], BF16)
                nc.vector.memset(zpadb, 0.0)
                # persistent conv x staging (zeroed once: the packed DMA only
                # writes 16 of every 32 cols; the transpose reads all 128)
                xrp = [cn.tile([128, 128], F32, name=f"xrp{i}") for i in range(4)]
                for i in range(4):
                    nc.vector.memset(xrp[i], 0.0)

                # persistent state
                c2 = cn.tile([128, 256], F32)         # cell, stacked
                nc.vector.memset(c2, 0.0)
                hTb = [cn.tile([128, 256], BF16, name=f"hT{i}") for i in range(2)]
                outF = cn.tile([B, OUT_STEPS], F32)
                dcol = cn.tile([B, 1], F32)
                pcol = cn.tile([B, 1], F32)
                dif = cn.tile([B, 1], F32)
                difb = cn.tile([B, 1], BF16)
                corr_row = cn.tile([1, B], BF16)
                xb = [cn.tile([B, OC], BF16, name=f"xb{i}") for i in range(NXB)]
                xsT = [cn.tile([OC + 1, B], BF16, name=f"xsT{i}") for i in range(NXB)]
                nc.sync.dma_start(out=dcol, in_=dstart_d[:, :])

                # ---------- weight prep ----------
                with (
                    tc.tile_pool(name="wtmp", bufs=3) as wt,
                    tc.tile_pool(name="wps", bufs=3, space="PSUM") as wps,
                ):
                    ones_st = wt.tile([1, B], F32, tag="ones", bufs=1)
                    nc.vector.memset(ones_st, 1.0)
                    nc.vector.tensor_copy(ones_row, ones_st)

                    def prep_whh(src_d, dst_tiles, dst_f32=False):
                        for jb in range(16):
                            n = PERM2N[jb]
                            wtmp = wt.tile([128, H], F32, tag="wtmp")
                            nc.sync.dma_start(out=wtmp, in_=src_d[128 * jb:128 * (jb + 1), :])
                            for kc in range(4):
                                wtp = wps.tile([128, 128], F32, tag="wtp")
                                nc.tensor.transpose(wtp, wtmp[:, 128 * kc:128 * (kc + 1)], identity)
                                dst = dst_tiles[kc][:, 128 * n:128 * (n + 1)]
                                if kc % 2 == 0:
                                    nc.scalar.copy(dst, wtp)
                                else:
                                    nc.vector.tensor_copy(dst, wtp)

                    prep_whh(encWhh_d, hW)

                    # decoder Whh.T staged fp32 for the fold, cast after
                    dhWs = [wt.tile([128, G], F32, tag=f"dhWs{k}", bufs=1,
                                    name=f"dhWs{k}") for k in range(4)]
                    prep_whh(decWhh_d, dhWs)

                    # enc_Wih.T into xW rows 0:64
                    for jb in range(16):
                        n = PERM2N[jb]
                        wtmp2 = wt.tile([128, OC], F32, tag="wtmp2")
                        nc.sync.dma_start(out=wtmp2, in_=encWih_d[128 * jb:128 * (jb + 1), :])
                        wtp = wps.tile([128, 128], F32, tag="wtp")
                        nc.tensor.transpose(wtp[:OC, :128], wtmp2, identity)
                        nc.scalar.copy(xW[0:OC, 128 * n:128 * (n + 1)], wtp[:OC, :128])

                    # bias rows / dWih rows in my col order (fp32 staging)
                    bstage = wt.tile([1, G], F32, tag="bstage", bufs=1)
                    dstage0 = wt.tile([1, G], F32, tag="dstage0", bufs=1)
                    dbstage = wt.tile([1, G], F32, tag="dbstage", bufs=1)
                    for n in range(16):
                        jb = PERM2N[n]
                        nc.sync.dma_start(out=bstage[:, 128 * n:128 * (n + 1)],
                                          in_=encb_d[None, 128 * jb:128 * (jb + 1)])
                        nc.sync.dma_start(out=dstage0[:, 128 * n:128 * (n + 1)],
                                          in_=decWih_d[128 * jb:128 * (jb + 1), :].rearrange("a b -> b a"))
                        nc.sync.dma_start(out=dbstage[:, 128 * n:128 * (n + 1)],
                                          in_=decb_d[None, 128 * jb:128 * (jb + 1)])
                    nc.vector.tensor_copy(xW[OC:OC + 1, :], bstage)

                    hwrow = wt.tile([1, H], F32, tag="hwrow", bufs=1)
                    nc.sync.dma_start(out=hwrow, in_=headw_d[:, :])
                    hbst = wt.tile([1, 1], F32, tag="hbst", bufs=1)
                    nc.sync.dma_start(out=hbst, in_=headb_d[:, None])
                    nc.vector.tensor_copy(hbr, hbst)
                    hwstage = wt.tile([128, 4], F32, tag="hwstage", bufs=1)
                    nc.sync.dma_start(out=hwstage,
                                      in_=headw_d[0].rearrange("(kc p) -> p kc", p=128))
                    nc.vector.tensor_copy(hwT, hwstage)

                    # fold: db' = db + head_b * dWih ; dxW0 = dWih
                    tmpb = wt.tile([1, G], F32, tag="tmpb", bufs=1)
                    dbrow = wt.tile([1, G], F32, tag="dbrow", bufs=1)
                    nc.vector.tensor_scalar_mul(tmpb, dstage0[:, :], hbst[0:1, 0:1])
                    nc.vector.tensor_add(dbrow, dbstage, tmpb)
                    nc.vector.tensor_copy(dxW0, dstage0)
                    nc.vector.tensor_copy(dbW, dbrow)

                    # dhW = bf16(dhWs + head_w (x) dWih)
                    for kc in range(4):
                        for n in range(4):
                            po = wps.tile([128, 512], F32, tag="po")
                            nc.tensor.matmul(
                                po,
                                lhsT=hwrow[:, 128 * kc:128 * (kc + 1)],
                                rhs=dstage0[:, 512 * n:512 * (n + 1)],
                                start=True, stop=True)
                            tfo = wt.tile([128, 512], F32, tag="tfo")
                            nc.vector.tensor_add(tfo, dhWs[kc][:, 512 * n:512 * (n + 1)], po)
                            if n % 2 == 0:
                                nc.scalar.copy(dhW[kc][:, 512 * n:512 * (n + 1)], tfo)
                            else:
                                nc.vector.tensor_copy(dhW[kc][:, 512 * n:512 * (n + 1)], tfo)

                    # conv weights: cwT[:, 64k:64k+64] = conv_w[:, :, k].T
                    cstage = wt.tile([CIN, KW * OC], F32, tag="cstage", bufs=1)
                    for k in range(KW):
                        nc.sync.dma_start(
                            out=cstage[:, OC * k:OC * (k + 1)],
                            in_=convw_d[:, :, k].rearrange("oc ic -> ic oc"),
                        )
                    nc.vector.tensor_copy(cwT, cstage)
                    nc.sync.dma_start(out=cb2[0:64, :], in_=convb_d[:, None])
                    nc.sync.dma_start(out=cb2[64:128, :], in_=convb_d[:, None])

                    for i in range(NXB):
                        nc.vector.tensor_copy(xsT[i][OC:OC + 1, :], ones_st)

                _mark(nc, "conv_start")
                # ---------- conv + pool -> enc_x (batch pairs) ----------
                with nc.named_scope("conv"):
                    with (
                        tc.tile_pool(name="conv", bufs=2) as cp,
                        tc.tile_pool(name="convps", bufs=2, space="PSUM") as cpp,
                    ):
                        for p in range(conv_pairs):
                            xTb = []
                            for j, b in enumerate((2 * p, 2 * p + 1)):
                                # rows 0:16 hold x[b].T (2-col zero pads); rows
                                # 16:32 scratch for the 32-aligned psum unpack.
                                xt = cp.tile([32, S + 4 + 4], BF16, tag=f"xTb{j}")
                                nc.vector.tensor_copy(xt[0:CIN, 0:2], zpadb)
                                nc.vector.tensor_copy(xt[0:CIN, 2 + S:2 + S + 2], zpadb)
                                for half in range(2):
                                    xr = xrp[(2 * j + half) % 4]
                                    nc.sync.dma_start(
                                        out=xr.rearrange("p (a c) -> p a c", c=32)[:, :, 0:CIN],
                                        in_=x_d[b].rearrange("(a p) c -> p a c", p=128)[:, 4 * half:4 * half + 4, :],
                                    )
                                    xtp = cpp.tile([128, 128], F32, tag="xtp")
                                    nc.tensor.transpose(xtp, xr, identity)
                                    for a in range(4):
                                        blk = xtp[32 * a:32 * (a + 1), :]
                                        dst = xt[:, 2 + 128 * (4 * half + a):2 + 128 * (4 * half + a + 1)]
                                        if a % 2 == 0:
                                            nc.scalar.copy(dst, blk)
                                        else:
                                            nc.vector.tensor_copy(dst, blk)
                                xTb.append(xt)
                            yb = cp.tile([128, S], BF16, tag="yb")
                            for hs in range(2):
                                cps = cpp.tile([128, 512], F32, tag="cps")
                                for k in range(KW):
                                    for j in range(2):
                                        nc.tensor.matmul(
                                            cps[64 * j:64 * (j + 1), :],
                                            lhsT=cwT[:, OC * k:OC * (k + 1)],
                                            rhs=xTb[j][0:CIN, k + 512 * hs:k + 512 * hs + 512],
                                            start=(k == 0), stop=(k == KW - 1),
                                            tile_position=(0, 64 * j),
                                            skip_group_check=True)
                                nc.scalar.activation(yb[:, 512 * hs:512 * (hs + 1)], cps,
                                                     AF.Relu, bias=cb2[:, 0:1])
                            pooled = cp.tile([128, T2], BF16, tag="pooled")
                            yp = yb.rearrange("p (t two) -> p t two", two=2)
                            nc.vector.tensor_max(pooled, yp[:, :, 0], yp[:, :, 1])
                            for q in range(4):
                                ptp = cpp.tile([128, 128], BF16, tag="ptp")
                                nc.tensor.transpose(ptp, pooled[:, 128 * q:128 * (q + 1)], idbf)
                                poolT = cp.tile([128, 128], BF16, tag="poolT")
                                if q % 2 == 0:
                                    nc.scalar.copy(poolT, ptp)
                                else:
                                    nc.vector.tensor_copy(poolT, ptp)
                                nc.sync.dma_start(
                                    out=enc_x[128 * q:128 * (q + 1), 2 * p:2 * p + 2, :],
                                    in_=poolT.rearrange("p (b oc) -> p b oc", b=2),
                                )

                _mark(nc, "enc_start")
                # ---------- encoder + decoder ----------
                with (
                    tc.tile_pool(name="step", bufs=2) as sp,
                    tc.tile_pool(name="lps", bufs=1, space="PSUM") as lp,
                ):
                    def prep_x_dma(t):
                        nc.sync.dma_start(out=xb[t % NXB], in_=enc_x[t])

                    def prep_x_tr(t):
                        xp = lp.tile([B, OC], BF16, tag="xps", bufs=2)
                        nc.tensor.transpose(xp, xb[t % NXB], idbf[0:64, 0:64])
                        nc.vector.tensor_copy(xsT[t % NXB][0:OC, :], xp)

                    def greg(gps, X, hf):
                        # gate X lives in bank X cols 0:256; rows split by half.
                        # one bank per gate: a start=True marks the whole 2KB
                        # bank row pending, so co-resident groups must be
                        # partition-disjoint only.
                        return gps[64 * hf:64 * (hf + 1), 512 * X:512 * X + 256]

                    def row_mms(gps, lhsT, Wrow, start, stop):
                        """K<=65 rank-ish updates: x-part / bias / correction."""
                        for X in range(4):
                            for hf in range(2):
                                nc.tensor.matmul(
                                    greg(gps, X, hf),
                                    lhsT=lhsT,
                                    rhs=Wrow[:, 512 * X + 256 * hf:512 * X + 256 * (hf + 1)],
                                    start=start, stop=stop,
                                    tile_position=(0, 64 * hf),
                                    skip_group_check=True)

                    def gate_mms(gps, W, hTt):
                        """h @ Whh.T, kc-major; col-group pairs run concurrent.
                        kc3 gate order (g,f,i,o) lets tanh(g) start early."""
                        for kc in range(4):
                            lhs = hTt[:, CMAP[kc]:CMAP[kc] + 64]
                            for X in ((3, 0, 1, 2) if kc == 3 else (0, 1, 2, 3)):
                                for hf in range(2):
                                    nc.tensor.matmul(
                                        greg(gps, X, hf),
                                        lhsT=lhs,
                                        rhs=W[kc][:, 512 * X + 256 * hf:512 * X + 256 * (hf + 1)],
                                        start=False, stop=(kc == 3),
                                        tile_position=(0, 64 * hf),
                                        skip_group_check=True)

                    def emit_htr(gps, h2t, dst):
                        """(h_lo, h_hi) [64,256] -> hT chunks [kc0|kc1|kc2|kc3].
                        f32 base-0 transposes land in the unused column halves
                        of the gate banks (emitted BEFORE the step's gate
                        groups start, so their start-marks can't clobber
                        partials). Base-64 row-group transposes hang the
                        device when interleaved with matmuls, hence the
                        base-0-only h_lo/h_hi split."""
                        h_lo, h_hi = h2t
                        nc.tensor.matmul(gps[:, 256:320], lhsT=h_lo[:, 0:128],
                                         rhs=identity[0:64, 0:64], is_transpose=True,
                                         skip_group_check=True)
                        nc.tensor.matmul(gps[:, 320:384], lhsT=h_lo[:, 128:256],
                                         rhs=identity[0:64, 0:64], is_transpose=True,
                                         skip_group_check=True)
                        nc.tensor.matmul(gps[:, 768:832], lhsT=h_hi[:, 0:128],
                                         rhs=identity[0:64, 0:64], is_transpose=True,
                                         skip_group_check=True)
                        nc.tensor.matmul(gps[:, 832:896], lhsT=h_hi[:, 128:256],
                                         rhs=identity[0:64, 0:64], is_transpose=True,
                                         skip_group_check=True)
                        nc.vector.tensor_copy(dst[:, 0:128], gps[:, 256:384])
                        nc.vector.tensor_copy(dst[:, 128:256], gps[:, 768:896])

                    def pred_mms(hTt, pp):
                        """pred = h @ head_w + head_b into psum [B, 1]."""
                        nc.tensor.matmul(pp, lhsT=ones_row, rhs=hbr,
                                         start=True, stop=False, tile_position=(0, 0))
                        for kc in range(4):
                            nc.tensor.matmul(pp, lhsT=hTt[:, CMAP[kc]:CMAP[kc] + 64],
                                             rhs=hwT[:, kc:kc + 1],
                                             start=False, stop=(kc == 3),
                                             tile_position=(0, 0))

                    def elementwise(gps, first):
                        sfio = sp.tile([128, 3 * 256], F32, tag="sfio")
                        tg = sp.tile([128, 256], F32, tag="tg")
                        tcl = sp.tile([128, 256], F32, tag="tc")
                        h_lo = sp.tile([64, 256], F32, tag="h_lo")
                        h_hi = sp.tile([64, 256], F32, tag="h_hi")
                        gv = gps.rearrange("p (bank c) -> p bank c", c=512)
                        nc.scalar.activation(sfio.rearrange("p (three c) -> p three c", three=3),
                                             gv[:, 0:3, 0:256], AF.Sigmoid)
                        nc.scalar.activation(tg, gps[:, 1536:1792], AF.Tanh)
                        if first:
                            nc.vector.tensor_mul(c2, sfio[:, 256:512], tg)
                        else:
                            t1 = sp.tile([128, 256], F32, tag="t1")
                            t2 = sp.tile([128, 256], F32, tag="t2")
                            nc.vector.tensor_mul(t1, sfio[:, 0:256], c2)
                            nc.vector.tensor_mul(t2, sfio[:, 256:512], tg)
                            nc.vector.tensor_add(c2, t1, t2)
                        nc.scalar.activation(tcl, c2, AF.Tanh)
                        nc.vector.tensor_mul(h_lo, sfio[0:64, 512:768], tcl[0:64, :])
                        nc.vector.tensor_mul(h_hi, sfio[64:128, 512:768], tcl[64:128, :])
                        return (h_lo, h_hi)

                    cur = 0
                    h2t = None
                    with nc.named_scope("enc"):
                        for t in range(min(3, t2)):
                            prep_x_dma(t)
                        prep_x_tr(0)
                        for t in range(t2):
                            if t + NXB - 1 < t2:
                                prep_x_dma(t + NXB - 1)
                            if t + 1 < t2:
                                prep_x_tr(t + 1)
                            gps = lp.tile([128, 2048], F32, tag="gates", bufs=1)
                            if t > 0:
                                emit_htr(gps, h2t, hTb[cur])
                            row_mms(gps, xsT[t % NXB], xW, True, t == 0)
                            if t > 0:
                                gate_mms(gps, hW, hTb[cur])
                            h2t = elementwise(gps, t == 0)
                            cur ^= 1

                    _mark(nc, "dec_start")
                    with nc.named_scope("dec"):
                        # pred_enc and the step-0 correction row
                        gpre = lp.tile([128, 2048], F32, tag="gates", bufs=1, name="gpre")
                        emit_htr(gpre, h2t, hTb[cur])
                        pp = lp.tile([B, 1], F32, tag="pred", bufs=2)
                        pred_mms(hTb[cur], pp)
                        nc.vector.tensor_copy(pcol, pp)
                        nc.vector.tensor_sub(dif, dcol, pcol)
                        nc.vector.tensor_copy(difb, dif)
                        ctp = lp.tile([B, OC], BF16, tag="xps", bufs=2, name="ctp")
                        nc.tensor.transpose(ctp[0:1, 0:B], difb, idbf[0:64, 0:64])
                        nc.vector.tensor_copy(corr_row, ctp[0:1, 0:B])

                        for d in range(steps):
                            gps = lp.tile([128, 2048], F32, tag="gates", bufs=1)
                            if d > 0:
                                cur ^= 1
                                emit_htr(gps, h2t, hTb[cur])
                            row_mms(gps, ones_row, dbW, True, False)
                            if d == 0:
                                row_mms(gps, corr_row, dxW0, False, False)
                            else:
                                pp = lp.tile([B, 1], F32, tag="pred", bufs=2)
                                pred_mms(hTb[cur], pp)
                                nc.vector.tensor_copy(outF[:, d - 1:d], pp)
                            gate_mms(gps, dhW, hTb[cur])
                            h2t = elementwise(gps, False)

                        cur ^= 1
                        gtail = lp.tile([128, 2048], F32, tag="gates", bufs=1, name="gtail")
                        emit_htr(gtail, h2t, hTb[cur])
                        pp = lp.tile([B, 1], F32, tag="pred", bufs=2)
                        pred_mms(hTb[cur], pp)
                        nc.vector.tensor_copy(outF[:, OUT_STEPS - 1:OUT_STEPS], pp)
                        nc.sync.dma_start(out=out_d[:, :], in_=outF)

    _mark(nc, "end")
    nc.compile()
    return nc


_CACHED = {}


def kernel(**inputs):
    """Full-input entry: shard batch across 8 cores, run SPMD, gather."""
    from concourse.bass_utils import run_bass_kernel_spmd

    if "nc" not in _CACHED:
        _CACHED["nc"] = build_nc()
    nc = _CACHED["nc"]

    full = {k: np.ascontiguousarray(np.asarray(v, dtype=np.float32)) for k, v in inputs.items()}
    per_core = []
    for c in range(NCORES):
        sl = slice(c * B, (c + 1) * B)
        m = {}
        for k, v in full.items():
            if k in ("x", "decoder_start"):
                m[k] = np.ascontiguousarray(v[sl])
            else:
                m[k] = v
        per_core.append(m)

    res = run_bass_kernel_spmd(nc, per_core, core_ids=list(range(NCORES)))
    outs = [r["out"] for r in res.results]
    return np.concatenate(outs, axis=0)
